# revision 1
# baseline (speedup 1.0000x reference)
"""Trainium2 Bass kernel for CropSplit (SipMask-style crop + quadrant split).

Reference computation, per output pixel (y, x, n):
    inside = point (x, y) lies in box rois[n] = (x1, y1, x2, y2)
    cell   = which of the 2x2 ROI sub-cells the pixel falls in
    out[y, x, n] = inside ? data[cell, y, x, n] : 0

Strategy:
  - Shard along H across the 8 cores (25 rows each). Every ROI spans the
    whole image but each PIXEL is independent, so any spatial shard works;
    H keeps DMA transfers contiguous along the innermost N axis.
  - The roi-derived masks are tiny (O(N*(H+W))) and are computed on host in
    float32 with bit-identical arithmetic to the reference, then shipped to
    the device as uint8 masks:
        hx[w, n]  = (cx == 1)   quadrant column select
        nix[w, n] = NOT inside_x
        hy[h, n]  = (cy == 1)   quadrant row select
        niy[h, n] = NOT inside_y
  - On device, with tiles laid out [w -> partitions, (h, n) -> free]:
        cp(d0, hx, d1)     d0 = hx ? d1 : d0     (x-blend, top row pair)
        cp(d2, hx, d3)     d2 = hx ? d3 : d2     (x-blend, bottom row pair)
        cp(d0, hy, d2)     d0 = hy ? d2 : d0     (y-blend -> 4-way select)
        cp(d0, nix, 0)     zero outside the box in x
        cp(d0, niy, 0)     zero outside the box in y
    All five are 2-stream DVE ops (copy_predicated), the minimum for a
    4-way data-dependent select + mask. Masks that are constant along h
    use 0-stride broadcast views; masks constant along w (partitions) are
    materialized once per core via DMA partition-broadcast.
"""

import numpy as np

C = 2
CC = C * C
H = W = N = 200
NCORES = 8
HS = H // NCORES  # 25 rows per core

W_CHUNKS = [(0, 128), (128, 72)]
H_BLOCKS = [(0, 13), (13, 12)]
DATA_BUFS = 3

_cache: dict = {}


def _build_module():
    import concourse.bacc as bacc
    import concourse.mybir as mybir
    from concourse.tile import TileContext

    f32 = mybir.dt.float32
    u8 = mybir.dt.uint8

    nc = bacc.Bacc(trn_type="TRN2", debug=False, num_devices=NCORES)
    data = nc.dram_tensor("data", [CC, HS, W, N], f32, kind="ExternalInput")
    mx = nc.dram_tensor("mx", [2, W, N], u8, kind="ExternalInput")  # hx, not_in_x
    my = nc.dram_tensor("my", [2, HS, N], u8, kind="ExternalInput")  # hy, not_in_y
    out = nc.dram_tensor("out", [HS, W, N], f32, kind="ExternalOutput")

    with TileContext(nc) as tc:
        with (
            tc.tile_pool(name="masks", bufs=1) as mpool,
            tc.tile_pool(name="dpool", bufs=DATA_BUFS) as dpool,
        ):
            zeros = mpool.tile([128, 1], f32)
            nc.vector.memset(zeros[:], 0.0)

            # x-masks: [w -> partition, n -> free], per w-chunk
            hx_t = []
            nix_t = []
            for ci, (w0, pw) in enumerate(W_CHUNKS):
                t_hx = mpool.tile([pw, N], u8, tag=f"hx{ci}")
                t_nix = mpool.tile([pw, N], u8, tag=f"nix{ci}")
                nc.sync.dma_start(t_hx[:], mx[0, w0 : w0 + pw, :])
                nc.sync.dma_start(t_nix[:], mx[1, w0 : w0 + pw, :])
                hx_t.append(t_hx)
                nix_t.append(t_nix)

            # y-masks: constant across partitions -> DMA partition-broadcast
            hy_rep = mpool.tile([128, HS, N], u8)
            niy_rep = mpool.tile([128, HS, N], u8)
            nc.sync.dma_start(hy_rep[:], my[0:1, :, :].broadcast_to((128, HS, N)))
            nc.sync.dma_start(niy_rep[:], my[1:2, :, :].broadcast_to((128, HS, N)))

            for ci, (w0, pw) in enumerate(W_CHUNKS):
                for h0, hb in H_BLOCKS:
                    d = []
                    for k in range(CC):
                        t = dpool.tile([pw, hb, N], f32, tag=f"d{k}")
                        nc.sync.dma_start(
                            t[:],
                            data[k, h0 : h0 + hb, w0 : w0 + pw, :].transpose([1, 0, 2]),
                        )
                        d.append(t)
                    hxv = hx_t[ci][:, None, :].broadcast_to((pw, hb, N))
                    nixv = nix_t[ci][:, None, :].broadcast_to((pw, hb, N))
                    hyv = hy_rep[:pw, h0 : h0 + hb, :]
                    niyv = niy_rep[:pw, h0 : h0 + hb, :]
                    zv = zeros[:pw, :, None].broadcast_to((pw, hb, N))
                    nc.vector.copy_predicated(d[0][:], hxv, d[1][:])
                    nc.vector.copy_predicated(d[2][:], hxv, d[3][:])
                    nc.vector.copy_predicated(d[0][:], hyv, d[2][:])
                    nc.vector.copy_predicated(d[0][:], nixv, zv)
                    nc.vector.copy_predicated(d[0][:], niyv, zv)
                    nc.sync.dma_start(
                        out[h0 : h0 + hb, w0 : w0 + pw, :].transpose([1, 0, 2]),
                        d[0][:],
                    )
    nc.finalize()
    return nc


def _get_module():
    if "nc" not in _cache:
        _cache["nc"] = _build_module()
    return _cache["nc"]


def _host_masks(rois):
    """Masks in f32 arithmetic bit-identical to the reference, as uint8."""
    r = np.asarray(rois, dtype=np.float32)
    x1, y1, x2, y2 = r[:, 0], r[:, 1], r[:, 2], r[:, 3]
    two = np.float32(2.0)
    one = np.float32(1.0)

    xs = np.arange(W, dtype=np.float32)[:, None]  # (W, 1)
    cw = np.maximum(x2 - x1, one)[None, :]  # (1, N)
    fx = np.floor(two * (xs - x1[None, :]) / cw)
    hx = (fx >= 1.0).astype(np.uint8)  # clip(floor, 0, 1) == 1
    nix = (~((xs >= x1[None, :]) & (xs <= x2[None, :]))).astype(np.uint8)

    ys = np.arange(H, dtype=np.float32)[:, None]  # (H, 1)
    ch = np.maximum(y2 - y1, one)[None, :]
    fy = np.floor(two * (ys - y1[None, :]) / ch)
    hy = (fy >= 1.0).astype(np.uint8)
    niy = (~((ys >= y1[None, :]) & (ys <= y2[None, :]))).astype(np.uint8)

    return hx, nix, hy, niy


def _run(data, rois, trace=False):
    from concourse.bass_utils import run_bass_kernel_spmd

    data = np.ascontiguousarray(np.asarray(data, dtype=np.float32))
    hx, nix, hy, niy = _host_masks(rois)
    mx = np.stack([hx, nix])  # (2, W, N) uint8
    my_full_h = np.stack([hy, niy])  # (2, H, N) uint8

    in_maps = []
    for i in range(NCORES):
        sl = slice(i * HS, (i + 1) * HS)
        in_maps.append(
            {
                "data": np.ascontiguousarray(data[:, sl]),
                "mx": mx,
                "my": np.ascontiguousarray(my_full_h[:, sl]),
            }
        )

    nc = _get_module()
    last_err = None
    for _attempt in range(2):
        try:
            res = run_bass_kernel_spmd(
                nc, in_maps, core_ids=list(range(NCORES)), trace=trace
            )
            break
        except Exception as e:  # transient NRT device errors: retry once
            last_err = e
    else:
        raise last_err
    full = np.concatenate([r["out"] for r in res.results], axis=0)
    return np.asarray(full, dtype=np.float32), res


def kernel(data, rois):
    out, _ = _run(data, rois, trace=False)
    return out


# revision 2
# speedup vs baseline: 1.1576x; 1.1576x over previous
"""Trainium2 Bass kernel for CropSplit (SipMask-style crop + quadrant split).

Reference computation, per output pixel (y, x, n):
    inside = point (x, y) lies in box rois[n] = (x1, y1, x2, y2)
    cell   = which of the 2x2 ROI sub-cells the pixel falls in
    out[y, x, n] = inside ? data[cell, y, x, n] : 0

Strategy:
  - Shard along W across the 8 cores (25 columns each). Each output pixel is
    independent, so any spatial shard works; W-sharding with an
    [h -> partitions, (w, n) -> free] tile layout makes every DMA row a
    large CONTIGUOUS DRAM block (w,n are the two innermost axes), which is
    what the DMA engines and HBM want. (H-sharding was measured at only
    ~16 GB/s per SDMA engine: 800B chunks with 160KB strides.)
  - The roi-derived masks are tiny (O(N*(H+W))), computed on host in
    float32 with bit-identical arithmetic to the reference, shipped as
    uint8:
        hx[w, n]  = (cx == 1)   quadrant column select
        nix[w, n] = NOT inside_x
        hy[h, n]  = (cy == 1)   quadrant row select
        niy[h, n] = NOT inside_y
  - Per tile, 5 predicated DVE ops implement select + mask:
        cp(d0, hx, d1)     d0 = hx ? d1 : d0     (x-blend, cy=0 pair)
        cp(d2, hx, d3)     d2 = hx ? d3 : d2     (x-blend, cy=1 pair)
        cp(d0, hy, d2)     d0 = hy ? d2 : d0     (y-blend -> 4-way select)
        cp(d0, nix, 0)     zero outside the box in x
        cp(d0, niy, 0)     zero outside the box in y
    Masks constant along w use 0-stride broadcast views ([h,n] tiles);
    masks constant along h (partitions) are materialized once per core via
    DMA partition-broadcast.
  - DMA issue is split across both HWDGE sequencers (Sync for loads,
    Scalar for stores/masks) to halve descriptor-generation serialization.
"""

import numpy as np

C = 2
CC = C * C
H = W = N = 200
NCORES = 8
WS = W // NCORES  # 25 columns per core

H_CHUNKS = [(0, 128), (128, 72)]
W_BLOCKS = [(0, 13), (13, 12)]
DATA_BUFS = 3

_cache: dict = {}


def _build_module():
    import concourse.bacc as bacc
    import concourse.mybir as mybir
    from concourse.tile import TileContext

    f32 = mybir.dt.float32
    u8 = mybir.dt.uint8

    nc = bacc.Bacc(trn_type="TRN2", debug=False, num_devices=NCORES)
    data = nc.dram_tensor("data", [CC, H, WS, N], f32, kind="ExternalInput")
    mxs = nc.dram_tensor("mxs", [2, WS, N], u8, kind="ExternalInput")  # hx, nix slab
    my = nc.dram_tensor("my", [2, H, N], u8, kind="ExternalInput")  # hy, niy full
    out = nc.dram_tensor("out", [H, WS, N], f32, kind="ExternalOutput")

    with TileContext(nc) as tc:
        with (
            tc.tile_pool(name="masks", bufs=1) as mpool,
            tc.tile_pool(name="dpool", bufs=DATA_BUFS) as dpool,
        ):
            zeros = mpool.tile([128, 1], f32)
            nc.vector.memset(zeros[:], 0.0)

            # y-masks: [h -> partition, n -> free], per h-chunk
            hy_t = []
            niy_t = []
            for ci, (h0, ph) in enumerate(H_CHUNKS):
                t_hy = mpool.tile([ph, N], u8, tag=f"hy{ci}")
                t_niy = mpool.tile([ph, N], u8, tag=f"niy{ci}")
                nc.scalar.dma_start(t_hy[:], my[0, h0 : h0 + ph, :])
                nc.scalar.dma_start(t_niy[:], my[1, h0 : h0 + ph, :])
                hy_t.append(t_hy)
                niy_t.append(t_niy)

            # x-masks: constant across partitions -> DMA partition-broadcast
            hx_rep = mpool.tile([128, WS, N], u8)
            nix_rep = mpool.tile([128, WS, N], u8)
            nc.scalar.dma_start(hx_rep[:], mxs[0:1, :, :].broadcast_to((128, WS, N)))
            nc.scalar.dma_start(nix_rep[:], mxs[1:2, :, :].broadcast_to((128, WS, N)))

            for ci, (h0, ph) in enumerate(H_CHUNKS):
                for w0, wb in W_BLOCKS:
                    d = []
                    for k in range(CC):
                        t = dpool.tile([ph, wb, N], f32, tag=f"d{k}")
                        nc.sync.dma_start(
                            t[:], data[k, h0 : h0 + ph, w0 : w0 + wb, :]
                        )
                        d.append(t)
                    hxv = hx_rep[:ph, w0 : w0 + wb, :]
                    nixv = nix_rep[:ph, w0 : w0 + wb, :]
                    hyv = hy_t[ci][:, None, :].broadcast_to((ph, wb, N))
                    niyv = niy_t[ci][:, None, :].broadcast_to((ph, wb, N))
                    zv = zeros[:ph, :, None].broadcast_to((ph, wb, N))
                    nc.vector.copy_predicated(d[0][:], hxv, d[1][:])
                    nc.vector.copy_predicated(d[2][:], hxv, d[3][:])
                    nc.vector.copy_predicated(d[0][:], hyv, d[2][:])
                    nc.vector.copy_predicated(d[0][:], nixv, zv)
                    nc.vector.copy_predicated(d[0][:], niyv, zv)
                    nc.scalar.dma_start(
                        out[h0 : h0 + ph, w0 : w0 + wb, :], d[0][:]
                    )
    nc.finalize()
    return nc


def _get_module():
    if "nc" not in _cache:
        _cache["nc"] = _build_module()
    return _cache["nc"]


def _host_masks(rois):
    """Masks in f32 arithmetic bit-identical to the reference, as uint8."""
    r = np.asarray(rois, dtype=np.float32)
    x1, y1, x2, y2 = r[:, 0], r[:, 1], r[:, 2], r[:, 3]
    two = np.float32(2.0)
    one = np.float32(1.0)

    xs = np.arange(W, dtype=np.float32)[:, None]  # (W, 1)
    cw = np.maximum(x2 - x1, one)[None, :]  # (1, N)
    fx = np.floor(two * (xs - x1[None, :]) / cw)
    hx = (fx >= 1.0).astype(np.uint8)  # clip(floor, 0, 1) == 1
    nix = (~((xs >= x1[None, :]) & (xs <= x2[None, :]))).astype(np.uint8)

    ys = np.arange(H, dtype=np.float32)[:, None]  # (H, 1)
    ch = np.maximum(y2 - y1, one)[None, :]
    fy = np.floor(two * (ys - y1[None, :]) / ch)
    hy = (fy >= 1.0).astype(np.uint8)
    niy = (~((ys >= y1[None, :]) & (ys <= y2[None, :]))).astype(np.uint8)

    return hx, nix, hy, niy


def _run(data, rois, trace=False):
    from concourse.bass_utils import run_bass_kernel_spmd

    data = np.ascontiguousarray(np.asarray(data, dtype=np.float32))
    hx, nix, hy, niy = _host_masks(rois)
    mx = np.stack([hx, nix])  # (2, W, N) uint8
    my = np.ascontiguousarray(np.stack([hy, niy]))  # (2, H, N) uint8

    in_maps = []
    for i in range(NCORES):
        sl = slice(i * WS, (i + 1) * WS)
        in_maps.append(
            {
                "data": np.ascontiguousarray(data[:, :, sl, :]),
                "mxs": np.ascontiguousarray(mx[:, sl, :]),
                "my": my,
            }
        )

    nc = _get_module()
    last_err = None
    for _attempt in range(2):
        try:
            res = run_bass_kernel_spmd(
                nc, in_maps, core_ids=list(range(NCORES)), trace=trace
            )
            break
        except Exception as e:  # transient NRT device errors: retry once
            last_err = e
    else:
        raise last_err
    full = np.concatenate([r["out"] for r in res.results], axis=1)
    return np.asarray(full, dtype=np.float32), res


def kernel(data, rois):
    out, _ = _run(data, rois, trace=False)
    return out
